# revision 10
# baseline (speedup 1.0000x reference)
"""ContextAttention Trainium2 kernel (8 NeuronCores).

Sharding: core i handles batch b=i//2, sequence half i%2 (2048 rows of N=4096).
All activations live transposed ([C, n] layout) so the contraction dim is on
partitions; per-(b,h) reductions over the full N are completed with a tiny
pairwise AllReduce between the two half-cores of each batch.

Math (per core, H=12 heads, D=64, C=768, n=2048 local rows):
  qkvT = qkv_w.T^T @ xT   (bf16, f32 psum)      [2304, n]
  delu(z) = relu(10z) + exp(10*min(z,0)) = max(10z,0) + min(exp(10z),1)
    -> 1 ACT op (Exp from psum, scale=10) + 1 DVE TS (max) +
       1 DVE scalar_tensor_tensor (min+add, fused accum -> ksum)
  kv/lkv diagonals: scalar_tensor_tensor (mult) reading v straight from
    PSUM with fused accum_out -> single DVE op each.
  AllReduce [ksum | kvd | lkvd] over the half pair; q tiles overlap it.
  s[h,n] = sum_d q[hd,n]*ksum[hd] via block one-hot matmul;
  norm = reciprocal_approx_fast(s)  (~18 bits, 5x faster than reciprocal)
  t1 = q * norm (one-hot broadcast matmul + DVE mult)
  out1T = (p1T * kvd)^T @ t1 + b1  (diag(kvd) folded into weights)
"""

import numpy as np
import ml_dtypes

import concourse.bass as bass
import concourse.mybir as mybir
import concourse.tile as tile
from concourse import bacc
from concourse.bass_utils import run_bass_kernel_spmd

bf16 = ml_dtypes.bfloat16
dt = mybir.dt
AF = mybir.ActivationFunctionType
OP = mybir.AluOpType

P = 128
NS = 2048          # local sequence rows per core
C = 768
H = 12
D = 64
KT = 6             # C // P     (k tiles / q-m-tiles / proj tiles)
NCH = 4            # NS // 512  (matmul free-dim chunks)
FD = 512
EPS = 1e-10
SCALE = D ** -0.5  # 0.125
RG = [[0, 1], [2, 3], [4, 5], [6, 7]]

_CACHE = {}


def _build():
    nc = bacc.Bacc("TRN2", target_bir_lowering=False, debug=False, num_devices=8)

    xT_in = nc.dram_tensor("xT", [C, NS], dt.bfloat16, kind="ExternalInput").ap()
    yT_in = nc.dram_tensor("yT", [C, NS], dt.bfloat16, kind="ExternalInput").ap()
    wq_in = nc.dram_tensor("wq", [C, 3 * C], dt.bfloat16, kind="ExternalInput").ap()
    p1_in = nc.dram_tensor("p1", [C, C], dt.bfloat16, kind="ExternalInput").ap()
    p2_in = nc.dram_tensor("p2", [C, C], dt.bfloat16, kind="ExternalInput").ap()
    b1_in = nc.dram_tensor("b1", [P, KT], dt.float32, kind="ExternalInput").ap()
    b2_in = nc.dram_tensor("b2", [P, KT], dt.float32, kind="ExternalInput").ap()
    oh_in = nc.dram_tensor("oh", [H, C], dt.bfloat16, kind="ExternalInput").ap()
    xo_out = nc.dram_tensor("xo", [C, NS], dt.bfloat16, kind="ExternalOutput").ap()
    yo_out = nc.dram_tensor("yo", [C, NS], dt.bfloat16, kind="ExternalOutput").ap()

    xT3 = xT_in.rearrange("(o p) f -> p o f", p=P)
    yT3 = yT_in.rearrange("(o p) f -> p o f", p=P)
    wq3 = wq_in.rearrange("(o p) f -> p o f", p=P)
    p13 = p1_in.rearrange("(o p) f -> p o f", p=P)
    p23 = p2_in.rearrange("(o p) f -> p o f", p=P)

    with tile.TileContext(nc) as tc:
        with (
            tc.tile_pool(name="persist", bufs=1) as pp,
            tc.tile_pool(name="scratch", bufs=8) as scr,
            tc.tile_pool(name="dram", bufs=1, space="DRAM") as dram,
        ):
            ccin = dram.tile([P, 18], dt.float32)
            ccout = dram.tile([2, P, 18], dt.float32)
            qbf = pp.tile([P, KT, NS], dt.bfloat16)
            red = pp.tile([P, 18], dt.float32)
            gred2 = pp.tile([P, 2, 18], dt.float32)
            gred = pp.tile([P, 18], dt.float32)
            ksum_eps = pp.tile([P, KT], dt.float32)
            kvls = pp.tile([P, 2 * KT], dt.float32)
            lhsT3 = pp.tile([P, KT, H], dt.bfloat16)
            oh_sb = pp.tile([H, C], dt.bfloat16)
            b1_sb = pp.tile([P, KT], dt.float32)
            b2_sb = pp.tile([P, KT], dt.float32)
            p1 = pp.tile([P, KT, C], dt.bfloat16)
            p2 = pp.tile([P, KT, C], dt.bfloat16)
            p1f = pp.tile([P, KT, C], dt.bfloat16)
            p2f = pp.tile([P, KT, C], dt.bfloat16)

            # ---------------- phase A: qkv matmuls + delu + local reductions
            with (
                tc.tile_pool(name="phA", bufs=1) as pa,
                tc.tile_pool(name="psA", bufs=2, space="PSUM") as psA,
            ):
                xT = pa.tile([P, KT, NS], dt.bfloat16)
                yT = pa.tile([P, KT, NS], dt.bfloat16)
                wq = pa.tile([P, KT, 3 * C], dt.bfloat16)

                # DMA issue order == consumption order.  k-weight-block and
                # xT pairs first (first matmul needs only pair kk=0), yT
                # interleaved, v/q weight blocks next, projections last.
                for kk in range(KT):
                    nc.sync.dma_start(wq[:, kk, C:2 * C], wq3[:, kk, C:2 * C])
                    nc.sync.dma_start(xT[:, kk, :], xT3[:, kk, :])
                    if kk == 3:
                        nc.sync.dma_start(yT[:, 0, :], yT3[:, 0, :])
                for kk in range(KT):
                    nc.sync.dma_start(wq[:, kk, 2 * C:3 * C], wq3[:, kk, 2 * C:3 * C])
                nc.sync.dma_start(yT[:, 1, :], yT3[:, 1, :])
                nc.sync.dma_start(yT[:, 2, :], yT3[:, 2, :])
                for kk in range(KT):
                    nc.sync.dma_start(wq[:, kk, 0:C], wq3[:, kk, 0:C])
                for j in range(3, KT):
                    nc.sync.dma_start(yT[:, j, :], yT3[:, j, :])
                nc.sync.dma_start(oh_sb[:], oh_in[:])
                nc.sync.dma_start(b1_sb[:], b1_in[:])
                nc.sync.dma_start(b2_sb[:], b2_in[:])
                for kk in range(KT):
                    nc.sync.dma_start(p1[:, kk, :], p13[:, kk, :])
                    nc.sync.dma_start(p2[:, kk, :], p23[:, kk, :])

                def mm_tile(m):
                    """qkv output m-tile -> [128, NS] psum (f32)."""
                    ps = psA.tile([P, NS], dt.float32, tag="psA")
                    for kk in range(KT):
                        for ch in range(NCH):
                            nc.tensor.matmul(
                                ps[:, ch * FD:(ch + 1) * FD],
                                wq[:, kk, m * P:(m + 1) * P],
                                xT[:, kk, ch * FD:(ch + 1) * FD],
                                start=(kk == 0),
                                stop=(kk == KT - 1),
                            )
                    return ps

                def delu(src, out_ap, acc=None, scale=10.0, gp=False):
                    """delu = max(10z,0) + min(exp(10z),1); acc += sum (fused).

                    Exp on ACT; relu on ACT (psum src) or DVE (sbuf src); the
                    min+add either fused on DVE (scalar_tensor_tensor with
                    free-dim accum) or as two plain ops on idle GpSimd
                    (gp=True, lk path) to keep DVE under the PE rate.
                    """
                    e = scr.tile([P, NS], dt.bfloat16, tag="scr")
                    nc.scalar.activation(e[:], src, AF.Exp, scale=scale)
                    r = scr.tile([P, NS], dt.bfloat16, tag="scr")
                    if gp:
                        nc.vector.tensor_scalar(r[:], src, scale, 0.0,
                                                OP.mult, OP.max)
                        m = scr.tile([P, NS], dt.bfloat16, tag="scr")
                        nc.gpsimd.tensor_scalar_min(m[:], e[:], 1.0)
                        nc.gpsimd.tensor_tensor(out_ap, r[:], m[:], OP.add)
                    else:
                        nc.scalar.activation(r[:], src, AF.Relu, scale=scale)
                        nc.vector.scalar_tensor_tensor(
                            out_ap, e[:], 1.0, r[:], OP.min, OP.add,
                            accum_out=acc)

                for j in range(KT):
                    ps_k = mm_tile(6 + j)
                    kbf = scr.tile([P, NS], dt.bfloat16, tag="kbf")
                    delu(ps_k[:], kbf[:], acc=red[:, j:j + 1])
                    # lk path has no psum dependency; runs under the v matmul
                    # on the otherwise-idle GpSimd engine
                    lkbf = scr.tile([P, NS], dt.bfloat16, tag="kbf")
                    delu(yT[:, j, :], lkbf[:], gp=True)
                    ps_v = mm_tile(12 + j)
                    # diagonals: delu(k)*v and delu(lk)*v, v read from PSUM,
                    # free-dim sums fused into the same instruction
                    pk = scr.tile([P, NS], dt.bfloat16, tag="scr")
                    nc.vector.scalar_tensor_tensor(
                        pk[:], kbf[:], 1.0, ps_v[:], OP.mult, OP.mult,
                        accum_out=red[:, 6 + j:7 + j])
                    pl = scr.tile([P, NS], dt.bfloat16, tag="scr")
                    nc.vector.scalar_tensor_tensor(
                        pl[:], lkbf[:], 1.0, ps_v[:], OP.mult, OP.mult,
                        accum_out=red[:, 12 + j:13 + j])

                # pairwise exchange of [ksum | kvd | lkvd] with the other
                # half-core.  AllGather + local add instead of AllReduce:
                # AllGather walks half the ncfw ring steps (N-1 vs 2N-2),
                # and the 2-slot add is one tiny DVE op.
                nc.gpsimd.dma_start(ccin[:], red[:])
                nc.gpsimd.collective_compute(
                    "AllGather", OP.bypass, replica_groups=RG,
                    ins=[ccin.opt()], outs=[ccout.opt()],
                )
                nc.gpsimd.dma_start(gred2[:, 0, :], ccout[0])
                nc.gpsimd.dma_start(gred2[:, 1, :], ccout[1])
                nc.vector.tensor_tensor(gred[:], gred2[:, 0, :],
                                        gred2[:, 1, :], OP.add)

                # post-collective scalars + weight folds — overlap the q tiles
                nc.vector.tensor_scalar_add(ksum_eps[:], gred[:, 0:KT], EPS)
                nc.vector.tensor_scalar_mul(kvls[:], gred[:, KT:18], SCALE)
                nc.vector.memset(lhsT3[:], 0.0)
                for j in range(KT):
                    nc.vector.tensor_copy(lhsT3[0:64, j, 2 * j:2 * j + 1],
                                          ksum_eps[0:64, j:j + 1])
                    nc.vector.tensor_copy(lhsT3[64:128, j, 2 * j + 1:2 * j + 2],
                                          ksum_eps[64:128, j:j + 1])
                # q tiles run while the collective is in flight
                for j in range(KT):
                    ps_q = mm_tile(j)
                    delu(ps_q[:], qbf[:, j, :])

                # folds only feed the projections — emit after the seam path,
                # on ACT so they don't queue ahead of the t1 muls on DVE
                for kk in range(KT):
                    nc.scalar.mul(p1f[:, kk, :], p1[:, kk, :],
                                  kvls[:, kk:kk + 1])
                    nc.scalar.mul(p2f[:, kk, :], p2[:, kk, :],
                                  kvls[:, KT + kk:KT + kk + 1])

            # ---------------- phase B: norm, t1, projections
            with tc.tile_pool(name="phB", bufs=1) as pb:
                t1 = pb.tile([P, KT, NS], dt.bfloat16)
                snorm = pb.tile([H, NS], dt.float32)
                snorm_bf = pb.tile([H, NS], dt.bfloat16)

                with tc.tile_pool(name="psS", bufs=2, space="PSUM") as psS:
                    for ch in range(NCH):
                        cs = slice(ch * FD, (ch + 1) * FD)
                        ps_s = psS.tile([H, FD], dt.float32, tag="psS")
                        for j in range(KT):
                            nc.tensor.matmul(
                                ps_s[:],
                                lhsT3[:, j, :],
                                qbf[:, j, cs],
                                start=(j == 0),
                                stop=(j == KT - 1),
                            )
                        nc.vector.reciprocal_approx_fast(snorm[:, cs], ps_s[:])
                        nc.scalar.copy(snorm_bf[:, cs], snorm[:, cs])

                with tc.tile_pool(name="psB", bufs=2, space="PSUM") as psB:
                    for j in range(KT):
                        ps_bc = psB.tile([P, NS], dt.float32, tag="psB")
                        for ch in range(NCH):
                            cs = slice(ch * FD, (ch + 1) * FD)
                            nc.tensor.matmul(
                                ps_bc[:, cs],
                                oh_sb[:, j * P:(j + 1) * P],
                                snorm_bf[:, cs],
                                start=True, stop=True,
                            )
                            nc.vector.tensor_tensor(t1[:, j, cs], qbf[:, j, cs],
                                                    ps_bc[:, cs], OP.mult)

                with (
                    tc.tile_pool(name="psO", bufs=2, space="PSUM") as psO,
                    tc.tile_pool(name="outp", bufs=2) as outp,
                ):
                    for mo in range(KT):
                        for wf, bias, dst in ((p1f, b1_sb, xo_out),
                                              (p2f, b2_sb, yo_out)):
                            ps_o = psO.tile([P, NS], dt.float32, tag="psO")
                            for kk in range(KT):
                                for ch in range(NCH):
                                    nc.tensor.matmul(
                                        ps_o[:, ch * FD:(ch + 1) * FD],
                                        wf[:, kk, mo * P:(mo + 1) * P],
                                        t1[:, kk, ch * FD:(ch + 1) * FD],
                                        start=(kk == 0),
                                        stop=(kk == KT - 1),
                                    )
                            osb = outp.tile([P, NS], dt.bfloat16, tag="outp")
                            nc.scalar.activation(osb[:], ps_o[:], AF.Identity,
                                                 bias=bias[:, mo:mo + 1], scale=1.0)
                            nc.sync.dma_start(dst[mo * P:(mo + 1) * P, :], osb[:])

    nc.compile()
    return nc


def _get_nc():
    if "nc" not in _CACHE:
        _CACHE["nc"] = _build()
    return _CACHE["nc"]


def _make_in_maps(x, y, qkv_w, proj1_w, proj1_b, proj2_w, proj2_b):
    wq_np = np.ascontiguousarray(qkv_w.T).astype(bf16)
    p1_np = np.ascontiguousarray(proj1_w.T).astype(bf16)
    p2_np = np.ascontiguousarray(proj2_w.T).astype(bf16)
    b1_np = np.ascontiguousarray(np.asarray(proj1_b, np.float32).reshape(KT, P).T)
    b2_np = np.ascontiguousarray(np.asarray(proj2_b, np.float32).reshape(KT, P).T)
    oh_np = np.zeros((H, C), bf16)
    for j in range(KT):
        oh_np[2 * j, j * P:j * P + 64] = 1
        oh_np[2 * j + 1, j * P + 64:(j + 1) * P] = 1
    in_maps = []
    for core in range(8):
        b_, h_ = core // 2, core % 2
        sl = slice(h_ * NS, (h_ + 1) * NS)
        xT = np.ascontiguousarray(np.asarray(x)[b_, sl].T).astype(bf16)
        yT = np.ascontiguousarray(np.asarray(y)[b_, sl].T).astype(bf16)
        in_maps.append({"xT": xT, "yT": yT, "wq": wq_np, "p1": p1_np,
                        "p2": p2_np, "b1": b1_np, "b2": b2_np, "oh": oh_np})
    return in_maps


def _unshard(results, B, N):
    xo = np.empty((B, N, C), np.float32)
    yo = np.empty((B, N, C), np.float32)
    for core in range(8):
        b_, h_ = core // 2, core % 2
        sl = slice(h_ * NS, (h_ + 1) * NS)
        xo[b_, sl] = results[core]["xo"].astype(np.float32).T
        yo[b_, sl] = results[core]["yo"].astype(np.float32).T
    return xo, yo


def kernel(x, y, qkv_w, proj1_w, proj1_b, proj2_w, proj2_b):
    nc = _get_nc()
    in_maps = _make_in_maps(x, y, qkv_w, proj1_w, proj1_b, proj2_w, proj2_b)
    res = run_bass_kernel_spmd(nc, in_maps, list(range(8)))
    return _unshard(res.results, np.asarray(x).shape[0], np.asarray(x).shape[1])


# revision 13
# speedup vs baseline: 1.6739x; 1.6739x over previous
"""ContextAttention Trainium2 kernel (8 NeuronCores).

Sharding: core i handles batch b=i//2, sequence half i%2 (2048 rows of N=4096).
All activations live transposed ([C, n] layout) so the contraction dim is on
partitions; per-(b,h) reductions over the full N are completed with a tiny
pairwise AllReduce between the two half-cores of each batch.

Math (per core, H=12 heads, D=64, C=768, n=2048 local rows):
  qkvT = qkv_w.T^T @ xT   (bf16, f32 psum)      [2304, n]
  delu(z) = relu(10z) + exp(10*min(z,0)) = max(10z,0) + min(exp(10z),1)
    -> 1 ACT op (Exp from psum, scale=10) + 1 DVE TS (max) +
       1 DVE scalar_tensor_tensor (min+add, fused accum -> ksum)
  kv/lkv diagonals: scalar_tensor_tensor (mult) reading v straight from
    PSUM with fused accum_out -> single DVE op each.
  AllReduce [ksum | kvd | lkvd] over the half pair; q tiles overlap it.
  s[h,n] = sum_d q[hd,n]*ksum[hd] via block one-hot matmul;
  norm = reciprocal_approx_fast(s)  (~18 bits, 5x faster than reciprocal)
  t1 = q * norm (one-hot broadcast matmul + DVE mult)
  out1T = (p1T * kvd)^T @ t1 + b1  (diag(kvd) folded into weights)
"""

import numpy as np
import ml_dtypes

import concourse.bass as bass
import concourse.mybir as mybir
import concourse.tile as tile
from concourse import bacc
from concourse.bass_utils import run_bass_kernel_spmd

bf16 = ml_dtypes.bfloat16
dt = mybir.dt
AF = mybir.ActivationFunctionType
OP = mybir.AluOpType

P = 128
NS = 2048          # local sequence rows per core
C = 768
H = 12
D = 64
KT = 6             # C // P     (k tiles / q-m-tiles / proj tiles)
NCH = 4            # NS // 512  (matmul free-dim chunks)
FD = 512
EPS = 1e-10
SCALE = D ** -0.5  # 0.125
RG = [[0, 1], [2, 3], [4, 5], [6, 7]]

_CACHE = {}


def _build():
    nc = bacc.Bacc("TRN2", target_bir_lowering=False, debug=False, num_devices=8)

    xT_in = nc.dram_tensor("xT", [C, NS], dt.bfloat16, kind="ExternalInput").ap()
    yT_in = nc.dram_tensor("yT", [C, NS], dt.bfloat16, kind="ExternalInput").ap()
    wq_in = nc.dram_tensor("wq", [C, 3 * C], dt.bfloat16, kind="ExternalInput").ap()
    p1_in = nc.dram_tensor("p1", [C, C], dt.bfloat16, kind="ExternalInput").ap()
    p2_in = nc.dram_tensor("p2", [C, C], dt.bfloat16, kind="ExternalInput").ap()
    b1_in = nc.dram_tensor("b1", [P, KT], dt.float32, kind="ExternalInput").ap()
    b2_in = nc.dram_tensor("b2", [P, KT], dt.float32, kind="ExternalInput").ap()
    oh_in = nc.dram_tensor("oh", [H, C], dt.bfloat16, kind="ExternalInput").ap()
    xo_out = nc.dram_tensor("xo", [C, NS], dt.bfloat16, kind="ExternalOutput").ap()
    yo_out = nc.dram_tensor("yo", [C, NS], dt.bfloat16, kind="ExternalOutput").ap()

    xT3 = xT_in.rearrange("(o p) f -> p o f", p=P)
    yT3 = yT_in.rearrange("(o p) f -> p o f", p=P)
    wq3 = wq_in.rearrange("(o p) f -> p o f", p=P)
    p13 = p1_in.rearrange("(o p) f -> p o f", p=P)
    p23 = p2_in.rearrange("(o p) f -> p o f", p=P)

    with tile.TileContext(nc) as tc:
        with (
            tc.tile_pool(name="persist", bufs=1) as pp,
            tc.tile_pool(name="scratch", bufs=8) as scr,
            tc.tile_pool(name="dram", bufs=1, space="DRAM") as dram,
        ):
            ccin = dram.tile([P, 18], dt.float32)
            ccout = dram.tile([2, P, 18], dt.float32)
            qbf = pp.tile([P, KT, NS], dt.bfloat16)
            red = pp.tile([P, 18], dt.float32)
            gred2 = pp.tile([P, 2, 18], dt.float32)
            gred = pp.tile([P, 18], dt.float32)
            ksum_eps = pp.tile([P, KT], dt.float32)
            kvls = pp.tile([P, 2 * KT], dt.float32)
            lhsT3 = pp.tile([P, KT, H], dt.bfloat16)
            oh_sb = pp.tile([H, C], dt.bfloat16)
            b1_sb = pp.tile([P, KT], dt.float32)
            b2_sb = pp.tile([P, KT], dt.float32)
            p1 = pp.tile([P, KT, C], dt.bfloat16)
            p2 = pp.tile([P, KT, C], dt.bfloat16)
            p1f = pp.tile([P, KT, C], dt.bfloat16)
            p2f = pp.tile([P, KT, C], dt.bfloat16)

            # ---------------- phase A: qkv matmuls + delu + local reductions
            with (
                tc.tile_pool(name="phA", bufs=1) as pa,
                tc.tile_pool(name="psA", bufs=2, space="PSUM") as psA,
            ):
                xT = pa.tile([P, KT, NS], dt.bfloat16)
                yT = pa.tile([P, KT, NS], dt.bfloat16)
                wq = pa.tile([P, KT, 3 * C], dt.bfloat16)

                # DMA issue order == consumption order.  k-weight-block and
                # xT pairs first (first matmul needs only pair kk=0), yT
                # interleaved, v/q weight blocks next, projections last.
                for kk in range(KT):
                    nc.sync.dma_start(wq[:, kk, C:2 * C], wq3[:, kk, C:2 * C])
                    nc.sync.dma_start(xT[:, kk, :], xT3[:, kk, :])
                    if kk == 3:
                        nc.sync.dma_start(yT[:, 0, :], yT3[:, 0, :])
                for kk in range(KT):
                    nc.sync.dma_start(wq[:, kk, 2 * C:3 * C], wq3[:, kk, 2 * C:3 * C])
                nc.sync.dma_start(yT[:, 1, :], yT3[:, 1, :])
                nc.sync.dma_start(yT[:, 2, :], yT3[:, 2, :])
                for kk in range(KT):
                    nc.sync.dma_start(wq[:, kk, 0:C], wq3[:, kk, 0:C])
                for j in range(3, KT):
                    nc.sync.dma_start(yT[:, j, :], yT3[:, j, :])
                nc.sync.dma_start(oh_sb[:], oh_in[:])
                nc.sync.dma_start(b1_sb[:], b1_in[:])
                nc.sync.dma_start(b2_sb[:], b2_in[:])
                for kk in range(KT):
                    nc.sync.dma_start(p1[:, kk, :], p13[:, kk, :])
                    nc.sync.dma_start(p2[:, kk, :], p23[:, kk, :])

                def mm_tile(m):
                    """qkv output m-tile -> [128, NS] psum (f32)."""
                    ps = psA.tile([P, NS], dt.float32, tag="psA")
                    for kk in range(KT):
                        for ch in range(NCH):
                            nc.tensor.matmul(
                                ps[:, ch * FD:(ch + 1) * FD],
                                wq[:, kk, m * P:(m + 1) * P],
                                xT[:, kk, ch * FD:(ch + 1) * FD],
                                start=(kk == 0),
                                stop=(kk == KT - 1),
                            )
                    return ps

                def delu(src, out_ap, acc=None, scale=10.0):
                    """delu = max(10z,0) + min(exp(10z),1); acc += sum (fused).

                    Exp + Relu on ACT (the only engine with exp; relu rides
                    along since ACT has slack); one DVE scalar_tensor_tensor
                    does min+add with the free-dim sum fused into accum_out.
                    """
                    e = scr.tile([P, NS], dt.bfloat16, tag="scr")
                    nc.scalar.activation(e[:], src, AF.Exp, scale=scale)
                    r = scr.tile([P, NS], dt.bfloat16, tag="scr")
                    nc.scalar.activation(r[:], src, AF.Relu, scale=scale)
                    nc.vector.scalar_tensor_tensor(
                        out_ap, e[:], 1.0, r[:], OP.min, OP.add,
                        accum_out=acc)

                for j in range(KT):
                    ps_k = mm_tile(6 + j)
                    kbf = scr.tile([P, NS], dt.bfloat16, tag="kbf")
                    delu(ps_k[:], kbf[:], acc=red[:, j:j + 1])
                    # lk path has no psum dependency; runs under the v matmul
                    lkbf = scr.tile([P, NS], dt.bfloat16, tag="kbf")
                    delu(yT[:, j, :], lkbf[:])
                    ps_v = mm_tile(12 + j)
                    # diagonals: delu(k)*v and delu(lk)*v, v read from PSUM,
                    # free-dim sums fused into the same instruction
                    pk = scr.tile([P, NS], dt.bfloat16, tag="scr")
                    nc.vector.scalar_tensor_tensor(
                        pk[:], kbf[:], 1.0, ps_v[:], OP.mult, OP.mult,
                        accum_out=red[:, 6 + j:7 + j])
                    pl = scr.tile([P, NS], dt.bfloat16, tag="scr")
                    nc.vector.scalar_tensor_tensor(
                        pl[:], lkbf[:], 1.0, ps_v[:], OP.mult, OP.mult,
                        accum_out=red[:, 12 + j:13 + j])

                # pairwise exchange of [ksum | kvd | lkvd] with the other
                # half-core.  AllGather + local add instead of AllReduce:
                # AllGather walks half the ncfw ring steps (N-1 vs 2N-2),
                # and the 2-slot add is one tiny DVE op.
                nc.gpsimd.dma_start(ccin[:], red[:])
                nc.gpsimd.collective_compute(
                    "AllGather", OP.bypass, replica_groups=RG,
                    ins=[ccin.opt()], outs=[ccout.opt()],
                )
                nc.gpsimd.dma_start(gred2[:, 0, :], ccout[0])
                nc.gpsimd.dma_start(gred2[:, 1, :], ccout[1])
                nc.vector.tensor_tensor(gred[:], gred2[:, 0, :],
                                        gred2[:, 1, :], OP.add)

                # post-collective scalars + weight folds — overlap the q tiles
                nc.vector.tensor_scalar_add(ksum_eps[:], gred[:, 0:KT], EPS)
                nc.vector.tensor_scalar_mul(kvls[:], gred[:, KT:18], SCALE)
                nc.vector.memset(lhsT3[:], 0.0)
                for j in range(KT):
                    nc.vector.tensor_copy(lhsT3[0:64, j, 2 * j:2 * j + 1],
                                          ksum_eps[0:64, j:j + 1])
                    nc.vector.tensor_copy(lhsT3[64:128, j, 2 * j + 1:2 * j + 2],
                                          ksum_eps[64:128, j:j + 1])
                # q tiles run while the collective is in flight
                for j in range(KT):
                    ps_q = mm_tile(j)
                    delu(ps_q[:], qbf[:, j, :])

                # folds only feed the projections — on DVE (ACT is busy with
                # the q-tile Exp/Relu chain through the q window)
                for kk in range(KT):
                    nc.vector.tensor_scalar_mul(p1f[:, kk, :], p1[:, kk, :],
                                                kvls[:, kk:kk + 1])
                    nc.vector.tensor_scalar_mul(p2f[:, kk, :], p2[:, kk, :],
                                                kvls[:, KT + kk:KT + kk + 1])

            # ---------------- phase B: norm, t1, projections
            with tc.tile_pool(name="phB", bufs=1) as pb:
                t1 = pb.tile([P, KT, NS], dt.bfloat16)
                snorm = pb.tile([H, NS], dt.float32)
                snorm_bf = pb.tile([H, NS], dt.bfloat16)

                with tc.tile_pool(name="psS", bufs=2, space="PSUM") as psS:
                    for ch in range(NCH):
                        cs = slice(ch * FD, (ch + 1) * FD)
                        ps_s = psS.tile([H, FD], dt.float32, tag="psS")
                        for j in range(KT):
                            nc.tensor.matmul(
                                ps_s[:],
                                lhsT3[:, j, :],
                                qbf[:, j, cs],
                                start=(j == 0),
                                stop=(j == KT - 1),
                            )
                        nc.vector.reciprocal_approx_fast(snorm[:, cs], ps_s[:])
                        nc.scalar.copy(snorm_bf[:, cs], snorm[:, cs])

                with tc.tile_pool(name="psB", bufs=2, space="PSUM") as psB:
                    for j in range(KT):
                        ps_bc = psB.tile([P, NS], dt.float32, tag="psB")
                        for ch in range(NCH):
                            cs = slice(ch * FD, (ch + 1) * FD)
                            nc.tensor.matmul(
                                ps_bc[:, cs],
                                oh_sb[:, j * P:(j + 1) * P],
                                snorm_bf[:, cs],
                                start=True, stop=True,
                            )
                            nc.vector.tensor_tensor(t1[:, j, cs], qbf[:, j, cs],
                                                    ps_bc[:, cs], OP.mult)

                with (
                    tc.tile_pool(name="psO", bufs=2, space="PSUM") as psO,
                    tc.tile_pool(name="outp", bufs=2) as outp,
                ):
                    for mo in range(KT):
                        for wf, bias, dst in ((p1f, b1_sb, xo_out),
                                              (p2f, b2_sb, yo_out)):
                            ps_o = psO.tile([P, NS], dt.float32, tag="psO")
                            for kk in range(KT):
                                for ch in range(NCH):
                                    nc.tensor.matmul(
                                        ps_o[:, ch * FD:(ch + 1) * FD],
                                        wf[:, kk, mo * P:(mo + 1) * P],
                                        t1[:, kk, ch * FD:(ch + 1) * FD],
                                        start=(kk == 0),
                                        stop=(kk == KT - 1),
                                    )
                            osb = outp.tile([P, NS], dt.bfloat16, tag="outp")
                            nc.scalar.activation(osb[:], ps_o[:], AF.Identity,
                                                 bias=bias[:, mo:mo + 1], scale=1.0)
                            nc.sync.dma_start(dst[mo * P:(mo + 1) * P, :], osb[:])

    nc.compile()
    return nc


def _get_nc():
    if "nc" not in _CACHE:
        _CACHE["nc"] = _build()
    return _CACHE["nc"]


def _make_in_maps(x, y, qkv_w, proj1_w, proj1_b, proj2_w, proj2_b):
    wq_np = np.ascontiguousarray(qkv_w.T).astype(bf16)
    p1_np = np.ascontiguousarray(proj1_w.T).astype(bf16)
    p2_np = np.ascontiguousarray(proj2_w.T).astype(bf16)
    b1_np = np.ascontiguousarray(np.asarray(proj1_b, np.float32).reshape(KT, P).T)
    b2_np = np.ascontiguousarray(np.asarray(proj2_b, np.float32).reshape(KT, P).T)
    oh_np = np.zeros((H, C), bf16)
    for j in range(KT):
        oh_np[2 * j, j * P:j * P + 64] = 1
        oh_np[2 * j + 1, j * P + 64:(j + 1) * P] = 1
    in_maps = []
    for core in range(8):
        b_, h_ = core // 2, core % 2
        sl = slice(h_ * NS, (h_ + 1) * NS)
        xT = np.ascontiguousarray(np.asarray(x)[b_, sl].T).astype(bf16)
        yT = np.ascontiguousarray(np.asarray(y)[b_, sl].T).astype(bf16)
        in_maps.append({"xT": xT, "yT": yT, "wq": wq_np, "p1": p1_np,
                        "p2": p2_np, "b1": b1_np, "b2": b2_np, "oh": oh_np})
    return in_maps


def _unshard(results, B, N):
    xo = np.empty((B, N, C), np.float32)
    yo = np.empty((B, N, C), np.float32)
    for core in range(8):
        b_, h_ = core // 2, core % 2
        sl = slice(h_ * NS, (h_ + 1) * NS)
        xo[b_, sl] = results[core]["xo"].astype(np.float32).T
        yo[b_, sl] = results[core]["yo"].astype(np.float32).T
    return xo, yo


def kernel(x, y, qkv_w, proj1_w, proj1_b, proj2_w, proj2_b):
    nc = _get_nc()
    in_maps = _make_in_maps(x, y, qkv_w, proj1_w, proj1_b, proj2_w, proj2_b)
    res = run_bass_kernel_spmd(nc, in_maps, list(range(8)))
    return _unshard(res.results, np.asarray(x).shape[0], np.asarray(x).shape[1])


# revision 18
# speedup vs baseline: 1.7431x; 1.0414x over previous
"""ContextAttention Trainium2 kernel (8 NeuronCores).

Sharding: core i handles batch b=i//2, sequence half i%2 (2048 rows of N=4096).
All activations live transposed ([C, n] layout) so the contraction dim is on
partitions; per-(b,h) reductions over the full N are completed with a tiny
pairwise AllReduce between the two half-cores of each batch.

Math (per core, H=12 heads, D=64, C=768, n=2048 local rows):
  qkvT = qkv_w.T^T @ xT   (bf16, f32 psum)      [2304, n]
  delu(z) = relu(10z) + exp(10*min(z,0)) = max(10z,0) + min(exp(10z),1)
    -> 1 ACT op (Exp from psum, scale=10) + 1 DVE TS (max) +
       1 DVE scalar_tensor_tensor (min+add, fused accum -> ksum)
  kv/lkv diagonals: scalar_tensor_tensor (mult) reading v straight from
    PSUM with fused accum_out -> single DVE op each.
  AllReduce [ksum | kvd | lkvd] over the half pair; q tiles overlap it.
  s[h,n] = sum_d q[hd,n]*ksum[hd] via block one-hot matmul;
  norm = reciprocal_approx_fast(s)  (~18 bits, 5x faster than reciprocal)
  t1 = q * norm (one-hot broadcast matmul + DVE mult)
  out1T = (p1T * kvd)^T @ t1 + b1  (diag(kvd) folded into weights)
"""

import numpy as np
import ml_dtypes

import concourse.bass as bass
import concourse.mybir as mybir
import concourse.tile as tile
from concourse import bacc
from concourse.bass_utils import run_bass_kernel_spmd

bf16 = ml_dtypes.bfloat16
dt = mybir.dt
AF = mybir.ActivationFunctionType
OP = mybir.AluOpType

P = 128
NS = 2048          # local sequence rows per core
C = 768
H = 12
D = 64
KT = 6             # C // P     (k tiles / q-m-tiles / proj tiles)
NCH = 4            # NS // 512  (matmul free-dim chunks)
FD = 512
EPS = 1e-10
SCALE = D ** -0.5  # 0.125
RG = [[0, 1], [2, 3], [4, 5], [6, 7]]

_CACHE = {}


def _build():
    nc = bacc.Bacc("TRN2", target_bir_lowering=False, debug=False, num_devices=8)

    xT_in = nc.dram_tensor("xT", [C, NS], dt.bfloat16, kind="ExternalInput").ap()
    yT_in = nc.dram_tensor("yT", [C, NS], dt.bfloat16, kind="ExternalInput").ap()
    wq_in = nc.dram_tensor("wq", [C, 3 * C], dt.bfloat16, kind="ExternalInput").ap()
    p1_in = nc.dram_tensor("p1", [C, C], dt.bfloat16, kind="ExternalInput").ap()
    p2_in = nc.dram_tensor("p2", [C, C], dt.bfloat16, kind="ExternalInput").ap()
    b1_in = nc.dram_tensor("b1", [P, KT], dt.float32, kind="ExternalInput").ap()
    b2_in = nc.dram_tensor("b2", [P, KT], dt.float32, kind="ExternalInput").ap()
    oh_in = nc.dram_tensor("oh", [H, C], dt.bfloat16, kind="ExternalInput").ap()
    xo_out = nc.dram_tensor("xo", [C, NS], dt.bfloat16, kind="ExternalOutput").ap()
    yo_out = nc.dram_tensor("yo", [C, NS], dt.bfloat16, kind="ExternalOutput").ap()

    xT3 = xT_in.rearrange("(o p) f -> p o f", p=P)
    yT3 = yT_in.rearrange("(o p) f -> p o f", p=P)
    wq3 = wq_in.rearrange("(o p) f -> p o f", p=P)
    p13 = p1_in.rearrange("(o p) f -> p o f", p=P)
    p23 = p2_in.rearrange("(o p) f -> p o f", p=P)

    with tile.TileContext(nc) as tc:
        with (
            tc.tile_pool(name="persist", bufs=1) as pp,
            tc.tile_pool(name="scratch", bufs=8) as scr,
            tc.tile_pool(name="scrk", bufs=4) as scrk,
            tc.tile_pool(name="scrv", bufs=3) as scrv,
            tc.tile_pool(name="dram", bufs=1, space="DRAM") as dram,
        ):
            ccin = dram.tile([P, 18], dt.float32)
            ccout = dram.tile([2, P, 18], dt.float32)
            qbf = pp.tile([P, KT, NS], dt.bfloat16)
            red = pp.tile([P, 18], dt.float32)
            gred2 = pp.tile([P, 2, 18], dt.float32)
            gred = pp.tile([P, 18], dt.float32)
            ksum_eps = pp.tile([P, KT], dt.float32)
            kvls = pp.tile([P, 2 * KT], dt.float32)
            lhsT3 = pp.tile([P, KT, H], dt.bfloat16)
            oh_sb = pp.tile([H, C], dt.bfloat16)
            b1_sb = pp.tile([P, KT], dt.float32)
            b2_sb = pp.tile([P, KT], dt.float32)
            p1 = pp.tile([P, KT, C], dt.bfloat16)
            p2 = pp.tile([P, KT, C], dt.bfloat16)
            p1f = pp.tile([P, KT, C], dt.bfloat16)
            p2f = pp.tile([P, KT, C], dt.bfloat16)

            # ---------------- phase A: qkv matmuls + delu + local reductions
            with (
                tc.tile_pool(name="phA", bufs=1) as pa,
                tc.tile_pool(name="psA", bufs=2, space="PSUM") as psA,
            ):
                xT = pa.tile([P, KT, NS], dt.bfloat16)
                yT = pa.tile([P, KT, NS], dt.bfloat16)
                wq = pa.tile([P, KT, 3 * C], dt.bfloat16)

                # DMA issue order == consumption order.  k-weight-block and
                # xT pairs first (first matmul needs only pair kk=0), yT
                # interleaved, v/q weight blocks next, projections last.
                for kk in range(KT):
                    nc.sync.dma_start(wq[:, kk, C:2 * C], wq3[:, kk, C:2 * C])
                    nc.sync.dma_start(xT[:, kk, :], xT3[:, kk, :])
                    if kk == 3:
                        nc.sync.dma_start(yT[:, 0, :], yT3[:, 0, :])
                for kk in range(KT):
                    nc.sync.dma_start(wq[:, kk, 2 * C:3 * C], wq3[:, kk, 2 * C:3 * C])
                nc.sync.dma_start(yT[:, 1, :], yT3[:, 1, :])
                nc.sync.dma_start(yT[:, 2, :], yT3[:, 2, :])
                for kk in range(KT):
                    nc.sync.dma_start(wq[:, kk, 0:C], wq3[:, kk, 0:C])
                for j in range(3, KT):
                    nc.sync.dma_start(yT[:, j, :], yT3[:, j, :])
                nc.sync.dma_start(oh_sb[:], oh_in[:])
                nc.sync.dma_start(b1_sb[:], b1_in[:])
                nc.sync.dma_start(b2_sb[:], b2_in[:])
                for kk in range(KT):
                    nc.sync.dma_start(p1[:, kk, :], p13[:, kk, :])
                    nc.sync.dma_start(p2[:, kk, :], p23[:, kk, :])

                def mm_tile(m):
                    """qkv output m-tile -> [128, NS] psum (f32)."""
                    ps = psA.tile([P, NS], dt.float32, tag="psA")
                    for kk in range(KT):
                        for ch in range(NCH):
                            nc.tensor.matmul(
                                ps[:, ch * FD:(ch + 1) * FD],
                                wq[:, kk, m * P:(m + 1) * P],
                                xT[:, kk, ch * FD:(ch + 1) * FD],
                                start=(kk == 0),
                                stop=(kk == KT - 1),
                            )
                    return ps

                def delu(src, out_ap, acc=None, scale=10.0, relu_dve=False):
                    """delu = max(10z,0) + min(exp(10z),1); acc += sum (fused).

                    Exp on ACT (only engine with exp); relu on ACT (psum
                    sources — frees the PSUM bank fast) or DVE (sbuf lk path —
                    keeps ACT under the PE rate); one DVE scalar_tensor_tensor
                    does min+add with the free-dim sum fused into accum_out.
                    """
                    e = scr.tile([P, NS], dt.bfloat16, tag="scr")
                    nc.scalar.activation(e[:], src, AF.Exp, scale=scale)
                    r = scr.tile([P, NS], dt.bfloat16, tag="scr")
                    if relu_dve:
                        nc.vector.tensor_scalar(r[:], src, scale, 0.0,
                                                OP.mult, OP.max)
                    else:
                        nc.scalar.activation(r[:], src, AF.Relu, scale=scale)
                    nc.vector.scalar_tensor_tensor(
                        out_ap, e[:], 1.0, r[:], OP.min, OP.add,
                        accum_out=acc)

                for j in range(KT):
                    ps_k = mm_tile(6 + j)
                    kbf = scrk.tile([P, NS], dt.bfloat16, tag="kbf")
                    delu(ps_k[:], kbf[:], acc=red[:, j:j + 1])
                    ps_v = mm_tile(12 + j)
                    # v: single fast ACT copy so the PSUM bank frees in ~2us;
                    # the diagonal products then read SBUF at their leisure
                    vbf = scrv.tile([P, NS], dt.bfloat16, tag="vbf")
                    nc.scalar.copy(vbf[:], ps_v[:])
                    # lk path has no psum dependency; relu on DVE keeps ACT
                    # (exp_k, relu_k, v copy, exp_lk) under the PE rate
                    lkbf = scrk.tile([P, NS], dt.bfloat16, tag="kbf")
                    delu(yT[:, j, :], lkbf[:], relu_dve=True)
                    # diagonals: delu(k)*v and delu(lk)*v with the free-dim
                    # sums fused into the same instruction
                    pk = scr.tile([P, NS], dt.bfloat16, tag="scr")
                    nc.vector.scalar_tensor_tensor(
                        pk[:], kbf[:], 1.0, vbf[:], OP.mult, OP.mult,
                        accum_out=red[:, 6 + j:7 + j])
                    pl = scr.tile([P, NS], dt.bfloat16, tag="scr")
                    nc.vector.scalar_tensor_tensor(
                        pl[:], lkbf[:], 1.0, vbf[:], OP.mult, OP.mult,
                        accum_out=red[:, 12 + j:13 + j])

                # pairwise exchange of [ksum | kvd | lkvd] with the other
                # half-core.  AllGather + local add instead of AllReduce:
                # AllGather walks half the ncfw ring steps (N-1 vs 2N-2),
                # and the 2-slot add is one tiny DVE op.
                nc.gpsimd.dma_start(ccin[:], red[:])
                nc.gpsimd.collective_compute(
                    "AllGather", OP.bypass, replica_groups=RG,
                    ins=[ccin.opt()], outs=[ccout.opt()],
                )
                nc.gpsimd.dma_start(gred2[:, 0, :], ccout[0])
                nc.gpsimd.dma_start(gred2[:, 1, :], ccout[1])
                nc.vector.tensor_tensor(gred[:], gred2[:, 0, :],
                                        gred2[:, 1, :], OP.add)

                # post-collective scalars + weight folds — overlap the q tiles
                nc.vector.tensor_scalar_add(ksum_eps[:], gred[:, 0:KT], EPS)
                nc.vector.tensor_scalar_mul(kvls[:], gred[:, KT:18], SCALE)
                nc.vector.memset(lhsT3[:], 0.0)
                for j in range(KT):
                    nc.vector.tensor_copy(lhsT3[0:64, j, 2 * j:2 * j + 1],
                                          ksum_eps[0:64, j:j + 1])
                    nc.vector.tensor_copy(lhsT3[64:128, j, 2 * j + 1:2 * j + 2],
                                          ksum_eps[64:128, j:j + 1])
                # q tiles run while the collective is in flight
                for j in range(KT):
                    ps_q = mm_tile(j)
                    delu(ps_q[:], qbf[:, j, :])

                # folds only feed the projections — on DVE (ACT is busy with
                # the q-tile Exp/Relu chain through the q window)
                for kk in range(KT):
                    nc.vector.tensor_scalar_mul(p1f[:, kk, :], p1[:, kk, :],
                                                kvls[:, kk:kk + 1])
                    nc.vector.tensor_scalar_mul(p2f[:, kk, :], p2[:, kk, :],
                                                kvls[:, KT + kk:KT + kk + 1])

            # ---------------- phase B: norm, t1, projections
            with tc.tile_pool(name="phB", bufs=1) as pb:
                t1 = pb.tile([P, KT, NS], dt.bfloat16)
                snorm = pb.tile([H, NS], dt.float32)
                snorm_bf = pb.tile([H, NS], dt.bfloat16)

                with tc.tile_pool(name="psS", bufs=2, space="PSUM") as psS:
                    for ch in range(NCH):
                        cs = slice(ch * FD, (ch + 1) * FD)
                        ps_s = psS.tile([H, FD], dt.float32, tag="psS")
                        for j in range(KT):
                            nc.tensor.matmul(
                                ps_s[:],
                                lhsT3[:, j, :],
                                qbf[:, j, cs],
                                start=(j == 0),
                                stop=(j == KT - 1),
                            )
                        nc.vector.reciprocal_approx_fast(snorm[:, cs], ps_s[:])
                        nc.scalar.copy(snorm_bf[:, cs], snorm[:, cs])

                with tc.tile_pool(name="psB", bufs=2, space="PSUM") as psB:
                    for j in range(KT):
                        ps_bc = psB.tile([P, NS], dt.float32, tag="psB")
                        for ch in range(NCH):
                            cs = slice(ch * FD, (ch + 1) * FD)
                            nc.tensor.matmul(
                                ps_bc[:, cs],
                                oh_sb[:, j * P:(j + 1) * P],
                                snorm_bf[:, cs],
                                start=True, stop=True,
                            )
                            nc.vector.tensor_tensor(t1[:, j, cs], qbf[:, j, cs],
                                                    ps_bc[:, cs], OP.mult)

                with (
                    tc.tile_pool(name="psO", bufs=2, space="PSUM") as psO,
                    tc.tile_pool(name="outp", bufs=2) as outp,
                ):
                    for mo in range(KT):
                        for wf, bias, dst in ((p1f, b1_sb, xo_out),
                                              (p2f, b2_sb, yo_out)):
                            ps_o = psO.tile([P, NS], dt.float32, tag="psO")
                            for kk in range(KT):
                                for ch in range(NCH):
                                    nc.tensor.matmul(
                                        ps_o[:, ch * FD:(ch + 1) * FD],
                                        wf[:, kk, mo * P:(mo + 1) * P],
                                        t1[:, kk, ch * FD:(ch + 1) * FD],
                                        start=(kk == 0),
                                        stop=(kk == KT - 1),
                                    )
                            osb = outp.tile([P, NS], dt.bfloat16, tag="outp")
                            # last tile: chunked epilogue shortens the drain
                            nch = 4 if mo == KT - 1 else 1
                            for c in range(nch):
                                cs = slice(c * NS // nch, (c + 1) * NS // nch)
                                nc.scalar.activation(osb[:, cs], ps_o[:, cs],
                                                     AF.Identity,
                                                     bias=bias[:, mo:mo + 1],
                                                     scale=1.0)
                                nc.sync.dma_start(dst[mo * P:(mo + 1) * P, cs],
                                                  osb[:, cs])

    nc.compile()
    return nc


def _get_nc():
    if "nc" not in _CACHE:
        _CACHE["nc"] = _build()
    return _CACHE["nc"]


def _make_in_maps(x, y, qkv_w, proj1_w, proj1_b, proj2_w, proj2_b):
    wq_np = np.ascontiguousarray(qkv_w.T).astype(bf16)
    p1_np = np.ascontiguousarray(proj1_w.T).astype(bf16)
    p2_np = np.ascontiguousarray(proj2_w.T).astype(bf16)
    b1_np = np.ascontiguousarray(np.asarray(proj1_b, np.float32).reshape(KT, P).T)
    b2_np = np.ascontiguousarray(np.asarray(proj2_b, np.float32).reshape(KT, P).T)
    oh_np = np.zeros((H, C), bf16)
    for j in range(KT):
        oh_np[2 * j, j * P:j * P + 64] = 1
        oh_np[2 * j + 1, j * P + 64:(j + 1) * P] = 1
    in_maps = []
    for core in range(8):
        b_, h_ = core // 2, core % 2
        sl = slice(h_ * NS, (h_ + 1) * NS)
        xT = np.ascontiguousarray(np.asarray(x)[b_, sl].T).astype(bf16)
        yT = np.ascontiguousarray(np.asarray(y)[b_, sl].T).astype(bf16)
        in_maps.append({"xT": xT, "yT": yT, "wq": wq_np, "p1": p1_np,
                        "p2": p2_np, "b1": b1_np, "b2": b2_np, "oh": oh_np})
    return in_maps


def _unshard(results, B, N):
    xo = np.empty((B, N, C), np.float32)
    yo = np.empty((B, N, C), np.float32)
    for core in range(8):
        b_, h_ = core // 2, core % 2
        sl = slice(h_ * NS, (h_ + 1) * NS)
        xo[b_, sl] = results[core]["xo"].astype(np.float32).T
        yo[b_, sl] = results[core]["yo"].astype(np.float32).T
    return xo, yo


def kernel(x, y, qkv_w, proj1_w, proj1_b, proj2_w, proj2_b):
    nc = _get_nc()
    in_maps = _make_in_maps(x, y, qkv_w, proj1_w, proj1_b, proj2_w, proj2_b)
    res = run_bass_kernel_spmd(nc, in_maps, list(range(8)))
    return _unshard(res.results, np.asarray(x).shape[0], np.asarray(x).shape[1])


# revision 22
# speedup vs baseline: 1.7533x; 1.0058x over previous
"""ContextAttention Trainium2 kernel (8 NeuronCores).

Sharding: core i handles batch b=i//2, sequence half i%2 (2048 rows of N=4096).
All activations live transposed ([C, n] layout) so the contraction dim is on
partitions; per-(b,h) reductions over the full N are completed with a tiny
pairwise AllReduce between the two half-cores of each batch.

Math (per core, H=12 heads, D=64, C=768, n=2048 local rows):
  qkvT = qkv_w.T^T @ xT   (bf16, f32 psum)      [2304, n]
  delu(z) = relu(10z) + exp(10*min(z,0)) = max(10z,0) + min(exp(10z),1)
    -> 1 ACT op (Exp from psum, scale=10) + 1 DVE TS (max) +
       1 DVE scalar_tensor_tensor (min+add, fused accum -> ksum)
  kv/lkv diagonals: scalar_tensor_tensor (mult) reading v straight from
    PSUM with fused accum_out -> single DVE op each.
  AllReduce [ksum | kvd | lkvd] over the half pair; q tiles overlap it.
  s[h,n] = sum_d q[hd,n]*ksum[hd] via block one-hot matmul;
  norm = reciprocal_approx_fast(s)  (~18 bits, 5x faster than reciprocal)
  t1 = q * norm (one-hot broadcast matmul + DVE mult)
  out1T = (p1T * kvd)^T @ t1 + b1  (diag(kvd) folded into weights)
"""

import numpy as np
import ml_dtypes

import concourse.bass as bass
import concourse.mybir as mybir
import concourse.tile as tile
from concourse import bacc
from concourse.bass_utils import run_bass_kernel_spmd

bf16 = ml_dtypes.bfloat16
dt = mybir.dt
AF = mybir.ActivationFunctionType
OP = mybir.AluOpType

P = 128
NS = 2048          # local sequence rows per core
C = 768
H = 12
D = 64
KT = 6             # C // P     (k tiles / q-m-tiles / proj tiles)
NCH = 4            # NS // 512  (matmul free-dim chunks)
FD = 512
EPS = 1e-10
SCALE = D ** -0.5  # 0.125
RG = [[0, 1], [2, 3], [4, 5], [6, 7]]

_CACHE = {}


def _build():
    nc = bacc.Bacc("TRN2", target_bir_lowering=False, debug=False, num_devices=8)

    xT_in = nc.dram_tensor("xT", [C, NS], dt.bfloat16, kind="ExternalInput").ap()
    yT_in = nc.dram_tensor("yT", [C, NS], dt.bfloat16, kind="ExternalInput").ap()
    wq_in = nc.dram_tensor("wq", [C, 3 * C], dt.bfloat16, kind="ExternalInput").ap()
    p1_in = nc.dram_tensor("p1", [C, C], dt.bfloat16, kind="ExternalInput").ap()
    p2_in = nc.dram_tensor("p2", [C, C], dt.bfloat16, kind="ExternalInput").ap()
    b1_in = nc.dram_tensor("b1", [P, KT], dt.float32, kind="ExternalInput").ap()
    b2_in = nc.dram_tensor("b2", [P, KT], dt.float32, kind="ExternalInput").ap()
    oh_in = nc.dram_tensor("oh", [H, C], dt.bfloat16, kind="ExternalInput").ap()
    xo_out = nc.dram_tensor("xo", [C, NS], dt.bfloat16, kind="ExternalOutput").ap()
    yo_out = nc.dram_tensor("yo", [C, NS], dt.bfloat16, kind="ExternalOutput").ap()

    xT3 = xT_in.rearrange("(o p) f -> p o f", p=P)
    yT3 = yT_in.rearrange("(o p) f -> p o f", p=P)
    wq3 = wq_in.rearrange("(o p) f -> p o f", p=P)
    p13 = p1_in.rearrange("(o p) f -> p o f", p=P)
    p23 = p2_in.rearrange("(o p) f -> p o f", p=P)

    with tile.TileContext(nc) as tc:
        with (
            tc.tile_pool(name="persist", bufs=1) as pp,
            tc.tile_pool(name="scratch", bufs=8) as scr,
            tc.tile_pool(name="scrk", bufs=4) as scrk,
            tc.tile_pool(name="scrv", bufs=3) as scrv,
            tc.tile_pool(name="dram", bufs=1, space="DRAM") as dram,
        ):
            ccin = dram.tile([P, 18], dt.float32)
            ccout = dram.tile([2, P, 18], dt.float32)
            qbf = pp.tile([P, KT, NS], dt.bfloat16)
            red = pp.tile([P, 18], dt.float32)
            gred2 = pp.tile([P, 2, 18], dt.float32)
            gred = pp.tile([P, 18], dt.float32)
            ksum_eps = pp.tile([P, KT], dt.float32)
            kvls = pp.tile([P, 2 * KT], dt.float32)
            lhsT3 = pp.tile([P, KT, H], dt.bfloat16)
            oh_sb = pp.tile([H, C], dt.bfloat16)
            b1_sb = pp.tile([P, KT], dt.float32)
            b2_sb = pp.tile([P, KT], dt.float32)
            p1 = pp.tile([P, KT, C], dt.bfloat16)
            p2 = pp.tile([P, KT, C], dt.bfloat16)
            p1f = pp.tile([P, KT, C], dt.bfloat16)
            p2f = pp.tile([P, KT, C], dt.bfloat16)

            # ---------------- phase A: qkv matmuls + delu + local reductions
            with (
                tc.tile_pool(name="phA", bufs=1) as pa,
                tc.tile_pool(name="psA", bufs=2, space="PSUM") as psA,
            ):
                xT = pa.tile([P, KT, NS], dt.bfloat16)
                yT = pa.tile([P, KT, NS], dt.bfloat16)
                wq = pa.tile([P, KT, 3 * C], dt.bfloat16)

                # DMA issue order == consumption order.  k-weight-block and
                # xT pairs first (first matmul needs only pair kk=0), yT
                # interleaved, v/q weight blocks next, projections last.
                for kk in range(KT):
                    nc.sync.dma_start(wq[:, kk, C:2 * C], wq3[:, kk, C:2 * C])
                    nc.sync.dma_start(xT[:, kk, :], xT3[:, kk, :])
                    if kk == 3:
                        nc.sync.dma_start(yT[:, 0, :], yT3[:, 0, :])
                for kk in range(KT):
                    nc.sync.dma_start(wq[:, kk, 2 * C:3 * C], wq3[:, kk, 2 * C:3 * C])
                nc.sync.dma_start(yT[:, 1, :], yT3[:, 1, :])
                nc.sync.dma_start(yT[:, 2, :], yT3[:, 2, :])
                nc.sync.dma_start(oh_sb[:], oh_in[:])
                nc.sync.dma_start(b1_sb[:], b1_in[:])
                nc.sync.dma_start(b2_sb[:], b2_in[:])

                def gated_dma(gate_ap, dst_ap, src_ap):
                    """DMA that waits for phase-A j=0 (via a 1-elem WAW gate)
                    so late-needed weights don't steal ramp DMA bandwidth."""
                    nc.vector.tensor_copy(gate_ap, red[0:1, 0:1])
                    nc.sync.dma_start(dst_ap, src_ap)

                for j in range(3, KT):
                    gated_dma(yT[0:1, j, 0:1], yT[:, j, :], yT3[:, j, :])
                for kk in range(KT):
                    gated_dma(wq[0:1, kk, 0:1], wq[:, kk, 0:C],
                              wq3[:, kk, 0:C])
                for kk in range(KT):
                    gated_dma(p1[0:1, kk, 0:1], p1[:, kk, :], p13[:, kk, :])
                    gated_dma(p2[0:1, kk, 0:1], p2[:, kk, :], p23[:, kk, :])

                def mm_tile(m):
                    """qkv output m-tile -> [128, NS] psum (f32)."""
                    ps = psA.tile([P, NS], dt.float32, tag="psA")
                    for kk in range(KT):
                        for ch in range(NCH):
                            nc.tensor.matmul(
                                ps[:, ch * FD:(ch + 1) * FD],
                                wq[:, kk, m * P:(m + 1) * P],
                                xT[:, kk, ch * FD:(ch + 1) * FD],
                                start=(kk == 0),
                                stop=(kk == KT - 1),
                            )
                    return ps

                def delu(src, out_ap, acc=None, scale=10.0, relu_dve=False):
                    """delu = max(10z,0) + min(exp(10z),1); acc += sum (fused).

                    Exp on ACT (only engine with exp); relu on ACT (psum
                    sources — frees the PSUM bank fast) or DVE (sbuf lk path —
                    keeps ACT under the PE rate); one DVE scalar_tensor_tensor
                    does min+add with the free-dim sum fused into accum_out.
                    """
                    e = scr.tile([P, NS], dt.bfloat16, tag="scr")
                    nc.scalar.activation(e[:], src, AF.Exp, scale=scale)
                    r = scr.tile([P, NS], dt.bfloat16, tag="scr")
                    if relu_dve:
                        nc.vector.tensor_scalar(r[:], src, scale, 0.0,
                                                OP.mult, OP.max)
                    else:
                        nc.scalar.activation(r[:], src, AF.Relu, scale=scale)
                    nc.vector.scalar_tensor_tensor(
                        out_ap, e[:], 1.0, r[:], OP.min, OP.add,
                        accum_out=acc)

                for j in range(KT):
                    ps_k = mm_tile(6 + j)
                    kbf = scrk.tile([P, NS], dt.bfloat16, tag="kbf")
                    delu(ps_k[:], kbf[:], acc=red[:, j:j + 1])
                    ps_v = mm_tile(12 + j)
                    # v: single fast ACT copy so the PSUM bank frees in ~2us;
                    # the diagonal products then read SBUF at their leisure
                    vbf = scrv.tile([P, NS], dt.bfloat16, tag="vbf")
                    nc.scalar.copy(vbf[:], ps_v[:])
                    # diagonals: delu(k)*v and delu(lk)*v with the free-dim
                    # sums fused into the same instruction; pk first (its
                    # inputs are ready before the lk chain finishes)
                    pk = scr.tile([P, NS], dt.bfloat16, tag="scr")
                    nc.vector.scalar_tensor_tensor(
                        pk[:], kbf[:], 1.0, vbf[:], OP.mult, OP.mult,
                        accum_out=red[:, 6 + j:7 + j])
                    # lk path has no psum dependency; relu on DVE keeps ACT
                    # (exp_k, relu_k, v copy, exp_lk) under the PE rate
                    lkbf = scrk.tile([P, NS], dt.bfloat16, tag="kbf")
                    delu(yT[:, j, :], lkbf[:], relu_dve=True)
                    pl = scr.tile([P, NS], dt.bfloat16, tag="scr")
                    nc.vector.scalar_tensor_tensor(
                        pl[:], lkbf[:], 1.0, vbf[:], OP.mult, OP.mult,
                        accum_out=red[:, 12 + j:13 + j])

                # pairwise exchange of [ksum | kvd | lkvd] with the other
                # half-core.  AllGather + local add instead of AllReduce:
                # AllGather walks half the ncfw ring steps (N-1 vs 2N-2),
                # and the 2-slot add is one tiny DVE op.
                nc.gpsimd.dma_start(ccin[:], red[:])
                nc.gpsimd.collective_compute(
                    "AllGather", OP.bypass, replica_groups=RG,
                    ins=[ccin.opt()], outs=[ccout.opt()],
                )
                nc.gpsimd.dma_start(gred2[:, 0, :], ccout[0])
                nc.gpsimd.dma_start(gred2[:, 1, :], ccout[1])
                nc.vector.tensor_tensor(gred[:], gred2[:, 0, :],
                                        gred2[:, 1, :], OP.add)

                # post-collective scalars + weight folds — overlap the q tiles
                nc.vector.tensor_scalar_add(ksum_eps[:], gred[:, 0:KT], EPS)
                nc.vector.tensor_scalar_mul(kvls[:], gred[:, KT:18], SCALE)
                nc.vector.memset(lhsT3[:], 0.0)
                for j in range(KT):
                    nc.vector.tensor_copy(lhsT3[0:64, j, 2 * j:2 * j + 1],
                                          ksum_eps[0:64, j:j + 1])
                    nc.vector.tensor_copy(lhsT3[64:128, j, 2 * j + 1:2 * j + 2],
                                          ksum_eps[64:128, j:j + 1])
                # q tiles run while the collective is in flight
                for j in range(KT):
                    ps_q = mm_tile(j)
                    delu(ps_q[:], qbf[:, j, :])

                # folds only feed the projections — on DVE (ACT is busy with
                # the q-tile Exp/Relu chain through the q window)
                for kk in range(KT):
                    nc.vector.tensor_scalar_mul(p1f[:, kk, :], p1[:, kk, :],
                                                kvls[:, kk:kk + 1])
                    nc.vector.tensor_scalar_mul(p2f[:, kk, :], p2[:, kk, :],
                                                kvls[:, KT + kk:KT + kk + 1])

            # ---------------- phase B: norm, t1, projections
            with tc.tile_pool(name="phB", bufs=1) as pb:
                t1 = pb.tile([P, KT, NS], dt.bfloat16)
                snorm = pb.tile([H, NS], dt.float32)
                snorm_bf = pb.tile([H, NS], dt.bfloat16)

                with tc.tile_pool(name="psS", bufs=2, space="PSUM") as psS:
                    for ch in range(NCH):
                        cs = slice(ch * FD, (ch + 1) * FD)
                        ps_s = psS.tile([H, FD], dt.float32, tag="psS")
                        for j in range(KT):
                            nc.tensor.matmul(
                                ps_s[:],
                                lhsT3[:, j, :],
                                qbf[:, j, cs],
                                start=(j == 0),
                                stop=(j == KT - 1),
                            )
                        nc.vector.reciprocal_approx_fast(snorm[:, cs], ps_s[:])
                        nc.scalar.copy(snorm_bf[:, cs], snorm[:, cs])

                with tc.tile_pool(name="psB", bufs=2, space="PSUM") as psB:
                    for j in range(KT):
                        ps_bc = psB.tile([P, NS], dt.float32, tag="psB")
                        for ch in range(NCH):
                            cs = slice(ch * FD, (ch + 1) * FD)
                            nc.tensor.matmul(
                                ps_bc[:, cs],
                                oh_sb[:, j * P:(j + 1) * P],
                                snorm_bf[:, cs],
                                start=True, stop=True,
                            )
                            nc.vector.tensor_tensor(t1[:, j, cs], qbf[:, j, cs],
                                                    ps_bc[:, cs], OP.mult)

                HNS = NS // 2
                with (
                    tc.tile_pool(name="psO", bufs=4, space="PSUM") as psO,
                    tc.tile_pool(name="outp", bufs=4) as outp,
                ):
                    for mo in range(KT):
                        for wf, bias, dst in ((p1f, b1_sb, xo_out),
                                              (p2f, b2_sb, yo_out)):
                            # half-width psum tiles (2 banks each, 4 in
                            # flight) so the epilogue drains never gate the
                            # matmul stream
                            for hf in range(2):
                                ps_o = psO.tile([P, HNS], dt.float32,
                                                tag="psO")
                                for kk in range(KT):
                                    for ch in range(2):
                                        c0 = hf * HNS + ch * FD
                                        nc.tensor.matmul(
                                            ps_o[:, ch * FD:(ch + 1) * FD],
                                            wf[:, kk, mo * P:(mo + 1) * P],
                                            t1[:, kk, c0:c0 + FD],
                                            start=(kk == 0),
                                            stop=(kk == KT - 1),
                                        )
                                osb = outp.tile([P, HNS], dt.bfloat16,
                                                tag="outp")
                                nch = 2 if mo == KT - 1 else 1
                                for c in range(nch):
                                    cs = slice(c * HNS // nch,
                                               (c + 1) * HNS // nch)
                                    ds = slice(hf * HNS + c * HNS // nch,
                                               hf * HNS + (c + 1) * HNS // nch)
                                    nc.scalar.activation(osb[:, cs],
                                                         ps_o[:, cs],
                                                         AF.Identity,
                                                         bias=bias[:, mo:mo + 1],
                                                         scale=1.0)
                                    nc.sync.dma_start(
                                        dst[mo * P:(mo + 1) * P, ds],
                                        osb[:, cs])

    nc.compile()
    return nc


def _get_nc():
    if "nc" not in _CACHE:
        _CACHE["nc"] = _build()
    return _CACHE["nc"]


def _make_in_maps(x, y, qkv_w, proj1_w, proj1_b, proj2_w, proj2_b):
    wq_np = np.ascontiguousarray(qkv_w.T).astype(bf16)
    p1_np = np.ascontiguousarray(proj1_w.T).astype(bf16)
    p2_np = np.ascontiguousarray(proj2_w.T).astype(bf16)
    b1_np = np.ascontiguousarray(np.asarray(proj1_b, np.float32).reshape(KT, P).T)
    b2_np = np.ascontiguousarray(np.asarray(proj2_b, np.float32).reshape(KT, P).T)
    oh_np = np.zeros((H, C), bf16)
    for j in range(KT):
        oh_np[2 * j, j * P:j * P + 64] = 1
        oh_np[2 * j + 1, j * P + 64:(j + 1) * P] = 1
    in_maps = []
    for core in range(8):
        b_, h_ = core // 2, core % 2
        sl = slice(h_ * NS, (h_ + 1) * NS)
        xT = np.ascontiguousarray(np.asarray(x)[b_, sl].T).astype(bf16)
        yT = np.ascontiguousarray(np.asarray(y)[b_, sl].T).astype(bf16)
        in_maps.append({"xT": xT, "yT": yT, "wq": wq_np, "p1": p1_np,
                        "p2": p2_np, "b1": b1_np, "b2": b2_np, "oh": oh_np})
    return in_maps


def _unshard(results, B, N):
    xo = np.empty((B, N, C), np.float32)
    yo = np.empty((B, N, C), np.float32)
    for core in range(8):
        b_, h_ = core // 2, core % 2
        sl = slice(h_ * NS, (h_ + 1) * NS)
        xo[b_, sl] = results[core]["xo"].astype(np.float32).T
        yo[b_, sl] = results[core]["yo"].astype(np.float32).T
    return xo, yo


def kernel(x, y, qkv_w, proj1_w, proj1_b, proj2_w, proj2_b):
    nc = _get_nc()
    in_maps = _make_in_maps(x, y, qkv_w, proj1_w, proj1_b, proj2_w, proj2_b)
    res = run_bass_kernel_spmd(nc, in_maps, list(range(8)))
    return _unshard(res.results, np.asarray(x).shape[0], np.asarray(x).shape[1])
